# revision 5
# baseline (speedup 1.0000x reference)
"""MoE ConditionalFeedForward kernel for 8 trn2 NeuronCores.

Strategy: expert parallelism. E=8 experts == 8 cores, so core k owns expert k's
weights (w1[k], w3[k], w2[k]) and processes exactly the (token, slot) pairs
routed to expert k. Routing/gather/scatter run on host; the heavy compute
(3 x C x D x I MACs per core over 1.1 GB of weights) runs on device.

Device math per core (C = padded token capacity, D=2048, I=5632):
  phase 1: hT[i, c] = silu(sum_d w1T[d,i] xT[d,c]) * (sum_d w3T[d,i] xT[d,c])
           (PE matmuls with d on partitions; w1/w3 pre-transposed on host)
  phase 2: y[c, d]  = sum_i hT[i, c] * w2[i, d]
           (PE matmuls with i on partitions; w2 in natural layout)

All weights/activations stream as bf16 (1 PE cycle/row vs 4 for f32; half the
HBM traffic); PSUM accumulation is f32 and the output is f32.
"""

import numpy as np
import ml_dtypes

BF16 = ml_dtypes.bfloat16

# Problem dims (hardcoded per contract; kernel.py must be self-contained).
T, A, E, D, I = 1024, 2, 8, 2048, 5632
N_CORES = 8

_BUILD_CACHE = {}


def _pick_groups(ib):
    """Blocks-per-DMA for the phase-1 (w1/w3) and phase-2 (w2) weight streams."""
    g1 = 2 if ib % 2 == 0 else 1
    g2 = 4 if ib % 4 == 0 else (2 if ib % 2 == 0 else 1)
    return g1, g2


def _pick_npass(n_cc, nd512):
    """Split phase 2's D dim into npass passes so live PSUM banks <= 8."""
    for npass in (1, 2, 4, 8):
        if nd512 % npass == 0 and n_cc * (nd512 // npass) <= 8 and nd512 >= npass:
            return npass
    raise ValueError(f"no valid npass for n_cc={n_cc} nd512={nd512}")


def _build(cap, d=D, i_dim=I):
    """Build + compile the per-core Bass program for token capacity `cap`."""
    key = (cap, d, i_dim)
    if key in _BUILD_CACHE:
        return _BUILD_CACHE[key]

    import concourse.mybir as mybir
    import concourse.tile as tile
    from concourse import bacc

    dt = mybir.dt
    WDT = dt.bfloat16
    F32 = dt.float32

    db = d // 128          # d-chunks (contraction of phase 1)
    ib = i_dim // 128      # i-blocks (contraction of phase 2)
    g1, g2 = _pick_groups(ib)
    ng1, ng2 = ib // g1, ib // g2
    assert cap % 64 == 0 and cap <= 512
    n_cc = (cap + 127) // 128
    nd512 = d // 512
    npass = _pick_npass(n_cc, nd512)
    w = d // npass         # output columns per phase-2 pass
    nw = w // 512          # 512-col chunks per pass

    nc = bacc.Bacc("TRN2", target_bir_lowering=False, debug=False,
                   num_devices=N_CORES)

    xgt = nc.dram_tensor("xgt", [128, db * cap], WDT, kind="ExternalInput").ap()
    w1d = nc.dram_tensor("w1d", [ng1, 128, g1 * db * 128], WDT,
                         kind="ExternalInput").ap()
    w3d = nc.dram_tensor("w3d", [ng1, 128, g1 * db * 128], WDT,
                         kind="ExternalInput").ap()
    w2d = nc.dram_tensor("w2d", [npass * ng2, 128, g2 * w], WDT,
                         kind="ExternalInput").ap()
    y = nc.dram_tensor("y", [cap, d], F32, kind="ExternalOutput").ap()

    with tile.TileContext(nc) as tc:
        with (
            tc.tile_pool(name="xpool", bufs=1) as xpool,
            tc.tile_pool(name="w1pool", bufs=3) as w1pool,
            tc.tile_pool(name="w3pool", bufs=3) as w3pool,
            tc.tile_pool(name="w2pool", bufs=3) as w2pool,
            tc.tile_pool(name="hpool", bufs=1) as hpool,
            tc.tile_pool(name="spool", bufs=2) as spool,
            tc.tile_pool(name="opool", bufs=4) as opool,
        ):
            xg = xpool.tile([128, db * cap], WDT)
            nc.sync.dma_start(xg[:], xgt[:])
            h = hpool.tile([128, ib * cap], WDT)

            # ---- phase 1: hT blocks ----
            with tc.tile_pool(name="psA", bufs=2, space="PSUM") as psA:
                for g in range(ng1):
                    wt1 = w1pool.tile([128, g1 * db * 128], WDT, tag="w1")
                    nc.sync.dma_start(wt1[:], w1d[g])
                    wt3 = w3pool.tile([128, g1 * db * 128], WDT, tag="w3")
                    nc.sync.dma_start(wt3[:], w3d[g])
                    for s in range(g1):
                        b = g * g1 + s
                        ps1 = psA.tile([128, cap], F32, tag="ps1")
                        ps3 = psA.tile([128, cap], F32, tag="ps3")
                        for do in range(db):
                            lo = (s * db + do) * 128
                            nc.tensor.matmul(
                                ps1[:], wt1[:, lo:lo + 128],
                                xg[:, do * cap:(do + 1) * cap],
                                start=(do == 0), stop=(do == db - 1))
                        for do in range(db):
                            lo = (s * db + do) * 128
                            nc.tensor.matmul(
                                ps3[:], wt3[:, lo:lo + 128],
                                xg[:, do * cap:(do + 1) * cap],
                                start=(do == 0), stop=(do == db - 1))
                        sig = spool.tile([128, cap], F32, tag="sig")
                        nc.scalar.activation(
                            sig[:], ps1[:],
                            mybir.ActivationFunctionType.Sigmoid)
                        m1 = spool.tile([128, cap], F32, tag="m1")
                        nc.vector.tensor_mul(m1[:], sig[:], ps3[:])
                        nc.vector.tensor_mul(
                            h[:, b * cap:(b + 1) * cap], m1[:], ps1[:])

            # ---- phase 2: y = hT.T @ w2 ----
            with tc.tile_pool(name="psB", bufs=1, space="PSUM") as psB:
                for ph in range(npass):
                    po = {}
                    for cc in range(n_cc):
                        for dn in range(nw):
                            po[cc, dn] = psB.tile(
                                [128, 512], F32, tag=f"o{cc}_{dn}",
                                name=f"po{cc}_{dn}")
                    for g in range(ng2):
                        wt2 = w2pool.tile([128, g2 * w], WDT, tag="w2")
                        nc.sync.dma_start(wt2[:], w2d[ph * ng2 + g])
                        for s in range(g2):
                            b = g * g2 + s
                            for cc in range(n_cc):
                                c0 = cc * 128
                                cs = min(128, cap - c0)
                                for dn in range(nw):
                                    nc.tensor.matmul(
                                        po[cc, dn][:cs, :],
                                        h[:, b * cap + c0:b * cap + c0 + cs],
                                        wt2[:, s * w + dn * 512:
                                            s * w + dn * 512 + 512],
                                        start=(b == 0), stop=(b == ib - 1))
                    for cc in range(n_cc):
                        c0 = cc * 128
                        cs = min(128, cap - c0)
                        for dn in range(nw):
                            ot = opool.tile([128, 512], F32, tag="ot")
                            nc.vector.tensor_copy(ot[:cs, :], po[cc, dn][:cs, :])
                            nc.scalar.dma_start(
                                y[c0:c0 + cs,
                                  ph * w + dn * 512:ph * w + dn * 512 + 512],
                                ot[:cs, :])

    nc.compile()
    _BUILD_CACHE[key] = nc
    return nc


def _pack_w13(wk, d=D, i_dim=I):
    """Host-side relayout of a [I, D] w1/w3 matrix into the pre-transposed
    phase-1 device layout (see _build)."""
    db, ib = d // 128, i_dim // 128
    g1, _ = _pick_groups(ib)
    ng1 = ib // g1
    # [g, s, i_in, do, di] -> [g, di, s, do, i_in]
    return np.ascontiguousarray(
        wk.reshape(ng1, g1, 128, db, 128).transpose(0, 4, 1, 3, 2)
    ).reshape(ng1, 128, g1 * db * 128)


def _pack_w2(w2k, npass, d=D, i_dim=I):
    db, ib = d // 128, i_dim // 128
    _, g2 = _pick_groups(ib)
    ng2 = ib // g2
    w = d // npass
    # [g, s, i_in, ph, dcol] -> [ph, g, i_in, s, dcol]
    return np.ascontiguousarray(
        w2k.reshape(ng2, g2, 128, npass, w).transpose(3, 0, 2, 1, 4)
    ).reshape(npass * ng2, 128, g2 * w)


def _prepare(inputs):
    """Host routing + packing. Returns (nc, in_maps, scatter_info)."""
    x = np.asarray(inputs["x"])
    idx = np.asarray(inputs["expert_indices"])
    w1 = np.asarray(inputs["w1"])
    w2 = np.asarray(inputs["w2"])
    w3 = np.asarray(inputs["w3"])

    t, a = idx.shape
    d, i_dim = x.shape[1], w1.shape[1]
    db = d // 128

    # ---- host routing (the "all-to-all") ----
    flat = idx.reshape(-1).astype(np.int64)
    order = np.argsort(flat, kind="stable")          # pair ids grouped by expert
    counts = np.bincount(flat, minlength=E)
    starts = np.concatenate([[0], np.cumsum(counts)])
    cap = max(128, int(-(-counts.max() // 64) * 64))  # round up to mult of 64
    assert cap <= 512, f"capacity {cap} > 512 unsupported"
    n_cc = (cap + 127) // 128
    npass = _pick_npass(n_cc, d // 512)

    nc = _build(cap, d, i_dim)

    x_bf = x.astype(BF16)
    in_maps = []
    for k in range(E):
        sel = order[starts[k]:starts[k + 1]] // a      # token ids for expert k
        xg = np.zeros((cap, d), BF16)
        xg[:len(sel)] = x_bf[sel]
        # [c, d] -> [di, do, c]
        xgt = np.ascontiguousarray(
            xg.T.reshape(db, 128, cap).transpose(1, 0, 2)
        ).reshape(128, db * cap)
        w1d_ = _pack_w13(w1[k].astype(BF16), d, i_dim)
        w3d_ = _pack_w13(w3[k].astype(BF16), d, i_dim)
        w2d_ = _pack_w2(w2[k].astype(BF16), npass, d, i_dim)
        in_maps.append({"xgt": xgt, "w1d": w1d_, "w3d": w3d_, "w2d": w2d_})

    return nc, in_maps, (t, a, d, order, counts, starts)


def _scatter(results, scatter_info):
    t, a, d, order, counts, starts = scatter_info
    out_flat = np.zeros((t * a, d), np.float32)
    for k in range(E):
        n_k = int(counts[k])
        if n_k:
            out_flat[order[starts[k]:starts[k] + n_k]] = results[k]["y"][:n_k]
    return out_flat.reshape(t, a, d)


def kernel(**inputs):
    from concourse.bass_utils import run_bass_kernel_spmd

    nc, in_maps, scatter_info = _prepare(inputs)
    res = run_bass_kernel_spmd(nc, in_maps, core_ids=list(range(N_CORES)))
    return _scatter(res.results, scatter_info)
